# revision 20
# baseline (speedup 1.0000x reference)
"""Trainium2 Bass kernel for nn_DecodeSBP (keypoint heatmap decode).

Contract: kernel(x=[1,133,512,512] f32) -> [133,3] f32
  joints[k] = (4*xx, 4*yy, conf) if conf > 0.8 else (-4, -4, -1)
  where flat = argmax(sigmoid(x[0,k])), conf = sigmoid(max), yy = flat//512,
  xx = flat%512. sigmoid is monotonic so the argmax runs on raw logits.

Sharding: keypoint dim across 8 cores (17/core, core 7 zero-padded).

Per-core program (one full-data pass, hierarchical argmax):
  stream: one 1 MB DMA per keypoint (the last keypoint in four quarter
    DMAs); DVE reduce_max emits per-(partition, 512-chunk) maxes right
    behind each tile's completion semaphore, so the reduction trails the
    17.8 MB stream by only ~1 tile.
  finale (single group, once, after the last tile):
    TensorE-transpose the 4 chunk columns into one PSUM tile [17, 512]
    whose column c*128+p is chunkmax(p, c) (mid-stream dummy transposes
    keep the PE p-state warm); per keypoint, "mask >= gmax times
    reversed-rank-iota, reduce_max" picks the winning chunk in exact
    flat-rank tie order (matters: keypoint 111 has a duplicated fp32
    max); ONE indirect_dma_start gathers all 17 winning 2 KB chunks
    (row indices from SBUF) -- per-keypoint register DMAs would stall
    all 16 DMA queues ~1.1 us each; DVE max/max_index give the index
    within the gathered chunk (tie-safe there: column order == flat
    order). Chunk size == image width, so yy = rank and xx = index.
    A confidence-gated copy_predicated assembles the output.
  The PE identity and rank/row-base iotas are host-provided inputs,
  removing the gpsimd iota preamble from the critical path.

Measured: ~69 us HW exec (baseline 89.6 us): ~45 us streaming at
~390 GB/s, ~8 us fixed framework preamble, ~16 us finale tail.
"""

import sys
from contextlib import ExitStack

for _p in ("/opt/trn_rl_repo", "/opt/pypackages"):
    if _p not in sys.path:
        sys.path.append(_p)

import numpy as np

import concourse.bacc as bacc
import concourse.bass as bass
import concourse.tile as tile
from concourse import mybir
from concourse.bass_utils import run_bass_kernel_spmd

K = 17          # keypoints per core
NK = 133        # total keypoints
ROW = 262144    # 512*512
P = 128         # SBUF partitions
F = ROW // P    # 2048 free elems per partition
C = 4           # chunks per partition row
S = F // C     # 512 elems per chunk
W = 512
N_CORES = 8
TILES = (1,) * K   # one keypoint per stream tile

f32 = mybir.dt.float32
i32 = mybir.dt.int32
u32 = mybir.dt.uint32
Alu = mybir.AluOpType
Act = mybir.ActivationFunctionType
X = mybir.AxisListType.X

_NC_CACHE = None


def _build():
    nc = bacc.Bacc("TRN2", target_bir_lowering=False, debug=False)
    x_dram = nc.dram_tensor("x", [K, ROW], f32, kind="ExternalInput")
    ident_dram = nc.dram_tensor("ident", [P, P], f32, kind="ExternalInput")
    consts_dram = nc.dram_tensor("consts", [K, 513], f32,
                                 kind="ExternalInput")
    out_dram = nc.dram_tensor("out", [K, 3], f32, kind="ExternalOutput")

    x_pkf = x_dram.ap().rearrange("k (p f) -> p k f", f=F)      # [128, K, 2048]
    x_rows = x_dram.ap().rearrange("k (r s) -> (k r) s", s=S)   # [K*512, 512]

    with tile.TileContext(nc) as tc, ExitStack() as ctx:
        const_pool = ctx.enter_context(tc.tile_pool(name="const", bufs=1))
        in_pool = ctx.enter_context(
            tc.tile_pool(name="in", bufs=len(TILES)))
        small_pool = ctx.enter_context(tc.tile_pool(name="small", bufs=1))
        psum_pool = ctx.enter_context(
            tc.tile_pool(name="psum", bufs=1, space="PSUM"))

        # stream DMAs first so descriptors hit the queues ASAP; one DMA per
        # keypoint (fast completion sems keep DVE close behind the stream),
        # the last keypoint in two half-DMAs so its reduce lags even less.
        # Last LQ keypoints stream as quarter-DMAs in CHUNK-MAJOR order:
        # all chunk-0 quarters first, ..., chunk-3 last. Dependency tracking
        # is per-region, so transpose matmul c fires as soon as the chunk-c
        # quarters land -- only the chunk-3 transpose stays in the tail.
        LQ = 3
        KQ = K - LQ
        tiles_sb = []
        for k in range(K):
            t = in_pool.tile([P, F], f32, tag="xin")
            if k < KQ:
                nc.sync.dma_start(t[:], x_pkf[:, k, :])
            tiles_sb.append((k, t))
        for c in range(C):
            for k in range(KQ, K):
                t = tiles_sb[k][1]
                nc.sync.dma_start(t[:, c * S:(c + 1) * S],
                                  x_pkf[:, k, c * S:(c + 1) * S])

        ident = const_pool.tile([P, P], f32)
        nc.scalar.dma_start(ident[:], ident_dram.ap())
        consts = const_pool.tile([K, 513], f32)
        nc.scalar.dma_start(consts[:], consts_dram.ap())
        # riota_pc[k, c*128+p] = 512 - (4p + c): rank of chunk (p,c) in flat
        # order, reversed so the masked reduce_max picks the first occurrence.
        riota_pc = consts[:, 0:512]
        kiota = consts[:, 512:513]         # kiota[k] = 512*(k+1)

        out_sb = small_pool.tile([K, 3], f32)
        nc.vector.memset(out_sb[:, 0:2], -4.0)
        nc.vector.memset(out_sb[:, 2:3], -1.0)

        # per-(partition, chunk) maxes, chunk columns in keypoint-major order
        pmax = small_pool.tile([P, K * C], f32)
        psumT = psum_pool.tile([K, C * P], f32)
        pm3 = pmax[:].rearrange("p (k c) -> p k c", c=C)
        warm = psum_pool.tile([1, P], f32, tag="warm")
        for k, t in tiles_sb[:KQ]:
            t3 = t[:].rearrange("p (c s) -> p c s", s=S)        # [P, C, S]
            nc.vector.reduce_max(
                pmax[:, k * C:(k + 1) * C], t3[:, :, :], axis=X)
            if k in (9, 12):
                # keep the PE p-state warm for the finale transposes
                nc.tensor.matmul(warm[:], pm3[:, 0:1, 0], ident[:],
                                 is_transpose=True)
        for c in range(C):
            for k in range(KQ, K):
                t3 = tiles_sb[k][1][:].rearrange("p (c s) -> p c s", s=S)
                nc.vector.reduce_max(
                    pmax[:, k * C + c:k * C + c + 1],
                    t3[:, c:c + 1, :], axis=X)
        # transpose chunk columns -> psumT[k, c*128+p] = chunkmax(p, c)
        for c in range(C):
            nc.tensor.matmul(psumT[:, c * P:(c + 1) * P],
                             pm3[:, :, c], ident[:], is_transpose=True)

        # winning chunk per keypoint, in exact flat-rank tie order:
        # mask (>= gmax) * reversed-rank-iota, reduce_max -> 512 - rank
        gmax = small_pool.tile([K, 1], f32)
        nc.vector.reduce_max(gmax[:], psumT[:], axis=X)
        cand_p = small_pool.tile([K, C * P], f32)
        nc.vector.scalar_tensor_tensor(
            cand_p[:], in0=psumT[:], scalar=gmax[:], in1=riota_pc,
            op0=Alu.is_ge, op1=Alu.mult)
        rc = small_pool.tile([K, 1], f32)   # 512 - rank
        nc.vector.reduce_max(rc[:], cand_p[:], axis=X)
        # gather row = 512*k + rank = kiota - rc
        offs_i = small_pool.tile([K, 1], i32)
        nc.vector.scalar_tensor_tensor(
            offs_i[:], in0=rc[:], scalar=-1.0, in1=kiota,
            op0=Alu.mult, op1=Alu.add)

        # one gather for all 17 winning chunks
        grow = small_pool.tile([K, S], f32)
        nc.gpsimd.indirect_dma_start(
            out=grow[:], out_offset=None, in_=x_rows,
            in_offset=bass.IndirectOffsetOnAxis(ap=offs_i[:, 0:1], axis=0))

        # off-critical-path decode prep (runs while the gather is in flight)
        cand = small_pool.tile([K, 3], f32)
        nc.scalar.activation(cand[:, 2:3], gmax[:], Act.Sigmoid)
        # yy = rank (chunk size == W): 4*yy = 2048 - 4*rc
        nc.vector.tensor_scalar(cand[:, 1:2], rc[:], -4.0, 2048.0,
                                Alu.mult, Alu.add)
        valid = small_pool.tile([K, 1], f32)
        nc.vector.tensor_scalar(valid[:], cand[:, 2:3], 0.8, None, Alu.is_gt)
        vb3 = small_pool.tile([K, 3], i32)
        nc.vector.tensor_scalar(vb3[:], out_sb[:], 0.0, valid[:],
                                Alu.mult, Alu.add)
        nc.vector.copy_predicated(out_sb[:, 1:3], vb3[:, 1:3], cand[:, 1:3])

        # index within the winning chunk == xx
        jmax8 = small_pool.tile([K, 8], f32)
        nc.vector.max(jmax8[:], grow[:])
        jidx8 = small_pool.tile([K, 8], u32)
        nc.vector.max_index(jidx8[:], jmax8[:], grow[:])
        nc.vector.tensor_scalar(cand[:, 0:1], jidx8[:, 0:1], 4.0, None,
                                Alu.mult)
        nc.vector.copy_predicated(out_sb[:, 0:1], vb3[:, 0:1], cand[:, 0:1])
        nc.scalar.dma_start(out_dram.ap()[:, :], out_sb[:])

    nc.compile()
    return nc


def _get_nc():
    global _NC_CACHE
    if _NC_CACHE is None:
        _NC_CACHE = _build()
    return _NC_CACHE


def _shard(x: np.ndarray) -> list[dict[str, np.ndarray]]:
    xf = np.ascontiguousarray(np.asarray(x, dtype=np.float32).reshape(NK, ROW))
    ident = np.ascontiguousarray(np.eye(P, dtype=np.float32))
    riota_pc = np.zeros(512, np.float32)
    for c in range(C):
        for p in range(P):
            riota_pc[c * P + p] = S - (C * p + c)
    kiota = (float(S) * (np.arange(K) + 1)).astype(np.float32)
    consts = np.ascontiguousarray(np.concatenate(
        [np.tile(riota_pc, (K, 1)), kiota[:, None]], axis=1, dtype=np.float32))
    shards = []
    for c in range(N_CORES):
        lo = c * K
        s = xf[lo:min(lo + K, NK)]
        if s.shape[0] < K:
            s = np.concatenate(
                [s, np.zeros((K - s.shape[0], ROW), np.float32)], axis=0)
        shards.append({"x": np.ascontiguousarray(s),
                       "ident": ident, "consts": consts})
    return shards


def _run(x, trace=False, **kw):
    nc = _get_nc()
    res = run_bass_kernel_spmd(nc, _shard(x), core_ids=list(range(N_CORES)),
                               trace=trace, **kw)
    out = np.concatenate([r["out"] for r in res.results], axis=0)[:NK]
    return out.astype(np.float32), res


def kernel(x: np.ndarray) -> np.ndarray:
    out, _ = _run(x, trace=False)
    return out


# revision 21
# speedup vs baseline: 1.0841x; 1.0841x over previous
"""Trainium2 Bass kernel for nn_DecodeSBP (keypoint heatmap decode).

Contract: kernel(x=[1,133,512,512] f32) -> [133,3] f32
  joints[k] = (4*xx, 4*yy, conf) if conf > 0.8 else (-4, -4, -1)
  where flat = argmax(sigmoid(x[0,k])), conf = sigmoid(max), yy = flat//512,
  xx = flat%512. sigmoid is monotonic so the argmax runs on raw logits.

Sharding: keypoint dim across 8 cores (17/core, core 7 zero-padded).

Per-core program (one full-data pass, hierarchical argmax):
  stream: one 1 MB DMA per keypoint (the last keypoint in four quarter
    DMAs); DVE reduce_max emits per-(partition, 512-chunk) maxes right
    behind each tile's completion semaphore, so the reduction trails the
    17.8 MB stream by only ~1 tile.
  finale (single group, once, after the last tile):
    TensorE-transpose the 4 chunk columns into one PSUM tile [17, 512]
    whose column c*128+p is chunkmax(p, c) (mid-stream dummy transposes
    keep the PE p-state warm); per keypoint, "mask >= gmax times
    reversed-rank-iota, reduce_max" picks the winning chunk in exact
    flat-rank tie order (matters: keypoint 111 has a duplicated fp32
    max); ONE indirect_dma_start gathers all 17 winning 2 KB chunks
    (row indices from SBUF) -- per-keypoint register DMAs would stall
    all 16 DMA queues ~1.1 us each; DVE max/max_index give the index
    within the gathered chunk (tie-safe there: column order == flat
    order). Chunk size == image width, so yy = rank and xx = index.
    A confidence-gated copy_predicated assembles the output.
  The PE identity and rank/row-base iotas are host-provided inputs,
  removing the gpsimd iota preamble from the critical path.

Measured: ~69 us HW exec (baseline 89.6 us): ~45 us streaming at
~390 GB/s, ~8 us fixed framework preamble, ~16 us finale tail.
"""

import sys
from contextlib import ExitStack

for _p in ("/opt/trn_rl_repo", "/opt/pypackages"):
    if _p not in sys.path:
        sys.path.append(_p)

import numpy as np

import concourse.bacc as bacc
import concourse.bass as bass
import concourse.tile as tile
from concourse import mybir
from concourse.bass_utils import run_bass_kernel_spmd

K = 17          # keypoints per core
NK = 133        # total keypoints
ROW = 262144    # 512*512
P = 128         # SBUF partitions
F = ROW // P    # 2048 free elems per partition
C = 4           # chunks per partition row
S = F // C     # 512 elems per chunk
W = 512
N_CORES = 8
TILES = (1,) * K   # one keypoint per stream tile

f32 = mybir.dt.float32
i32 = mybir.dt.int32
u32 = mybir.dt.uint32
Alu = mybir.AluOpType
Act = mybir.ActivationFunctionType
X = mybir.AxisListType.X

_NC_CACHE = None


def _build():
    nc = bacc.Bacc("TRN2", target_bir_lowering=False, debug=False)
    x_dram = nc.dram_tensor("x", [K, ROW], f32, kind="ExternalInput")
    ident_dram = nc.dram_tensor("ident", [P, P], f32, kind="ExternalInput")
    consts_dram = nc.dram_tensor("consts", [K, 513], f32,
                                 kind="ExternalInput")
    out_dram = nc.dram_tensor("out", [K, 3], f32, kind="ExternalOutput")

    x_pkf = x_dram.ap().rearrange("k (p f) -> p k f", f=F)      # [128, K, 2048]
    x_rows = x_dram.ap().rearrange("k (r s) -> (k r) s", s=S)   # [K*512, 512]

    with tile.TileContext(nc) as tc, ExitStack() as ctx:
        const_pool = ctx.enter_context(tc.tile_pool(name="const", bufs=1))
        in_pool = ctx.enter_context(
            tc.tile_pool(name="in", bufs=len(TILES)))
        small_pool = ctx.enter_context(tc.tile_pool(name="small", bufs=1))
        psum_pool = ctx.enter_context(
            tc.tile_pool(name="psum", bufs=1, space="PSUM"))

        # stream DMAs first so descriptors hit the queues ASAP; one DMA per
        # keypoint (fast completion sems keep DVE close behind the stream),
        # the last keypoint in two half-DMAs so its reduce lags even less.
        # One DMA per keypoint (8 KB descriptors, fast completion sems);
        # only the last keypoint is quartered so the finale can chase its
        # final bytes closely -- wider quartering slows the stream (2 KB
        # descriptors quadruple the descriptor count).
        KQ = K - 1
        tiles_sb = []
        for k in range(K):
            t = in_pool.tile([P, F], f32, tag="xin")
            if k < KQ:
                nc.sync.dma_start(t[:], x_pkf[:, k, :])
            tiles_sb.append((k, t))
        for c in range(C):
            t = tiles_sb[KQ][1]
            nc.sync.dma_start(t[:, c * S:(c + 1) * S],
                              x_pkf[:, KQ, c * S:(c + 1) * S])

        ident = const_pool.tile([P, P], f32)
        nc.scalar.dma_start(ident[:], ident_dram.ap())
        consts = const_pool.tile([K, 513], f32)
        nc.scalar.dma_start(consts[:], consts_dram.ap())
        # riota_pc[k, c*128+p] = 512 - (4p + c): rank of chunk (p,c) in flat
        # order, reversed so the masked reduce_max picks the first occurrence.
        riota_pc = consts[:, 0:512]
        kiota = consts[:, 512:513]         # kiota[k] = 512*(k+1)

        out_sb = small_pool.tile([K, 3], f32)
        nc.vector.memset(out_sb[:, 0:2], -4.0)
        nc.vector.memset(out_sb[:, 2:3], -1.0)

        # per-(partition, chunk) maxes, chunk columns in keypoint-major order
        pmax = small_pool.tile([P, K * C], f32)
        psumT = psum_pool.tile([K, C * P], f32)
        pm3 = pmax[:].rearrange("p (k c) -> p k c", c=C)
        warm = psum_pool.tile([1, P], f32, tag="warm")
        for k, t in tiles_sb[:KQ]:
            t3 = t[:].rearrange("p (c s) -> p c s", s=S)        # [P, C, S]
            nc.vector.reduce_max(
                pmax[:, k * C:(k + 1) * C], t3[:, :, :], axis=X)
            if k in (9, 12):
                # keep the PE p-state warm for the finale transposes
                nc.tensor.matmul(warm[:], pm3[:, 0:1, 0], ident[:],
                                 is_transpose=True)
        for c in range(C):
            t3 = tiles_sb[KQ][1][:].rearrange("p (c s) -> p c s", s=S)
            nc.vector.reduce_max(
                pmax[:, KQ * C + c:KQ * C + c + 1],
                t3[:, c:c + 1, :], axis=X)
        # transpose chunk columns -> psumT[k, c*128+p] = chunkmax(p, c)
        for c in range(C):
            nc.tensor.matmul(psumT[:, c * P:(c + 1) * P],
                             pm3[:, :, c], ident[:], is_transpose=True)

        # winning chunk per keypoint, in exact flat-rank tie order:
        # mask (>= gmax) * reversed-rank-iota, reduce_max -> 512 - rank
        gmax = small_pool.tile([K, 1], f32)
        nc.vector.reduce_max(gmax[:], psumT[:], axis=X)
        cand_p = small_pool.tile([K, C * P], f32)
        nc.vector.scalar_tensor_tensor(
            cand_p[:], in0=psumT[:], scalar=gmax[:], in1=riota_pc,
            op0=Alu.is_ge, op1=Alu.mult)
        rc = small_pool.tile([K, 1], f32)   # 512 - rank
        nc.vector.reduce_max(rc[:], cand_p[:], axis=X)
        # gather row = 512*k + rank = kiota - rc
        offs_i = small_pool.tile([K, 1], i32)
        nc.vector.scalar_tensor_tensor(
            offs_i[:], in0=rc[:], scalar=-1.0, in1=kiota,
            op0=Alu.mult, op1=Alu.add)

        # one gather for all 17 winning chunks
        grow = small_pool.tile([K, S], f32)
        nc.gpsimd.indirect_dma_start(
            out=grow[:], out_offset=None, in_=x_rows,
            in_offset=bass.IndirectOffsetOnAxis(ap=offs_i[:, 0:1], axis=0))

        # off-critical-path decode prep (runs while the gather is in flight)
        cand = small_pool.tile([K, 3], f32)
        nc.scalar.activation(cand[:, 2:3], gmax[:], Act.Sigmoid)
        # yy = rank (chunk size == W): 4*yy = 2048 - 4*rc
        nc.vector.tensor_scalar(cand[:, 1:2], rc[:], -4.0, 2048.0,
                                Alu.mult, Alu.add)
        valid = small_pool.tile([K, 1], f32)
        nc.vector.tensor_scalar(valid[:], cand[:, 2:3], 0.8, None, Alu.is_gt)
        vb3 = small_pool.tile([K, 3], i32)
        nc.vector.tensor_scalar(vb3[:], out_sb[:], 0.0, valid[:],
                                Alu.mult, Alu.add)
        nc.vector.copy_predicated(out_sb[:, 1:3], vb3[:, 1:3], cand[:, 1:3])

        # index within the winning chunk == xx
        jmax8 = small_pool.tile([K, 8], f32)
        nc.vector.max(jmax8[:], grow[:])
        jidx8 = small_pool.tile([K, 8], u32)
        nc.vector.max_index(jidx8[:], jmax8[:], grow[:])
        nc.vector.tensor_scalar(cand[:, 0:1], jidx8[:, 0:1], 4.0, None,
                                Alu.mult)
        nc.vector.copy_predicated(out_sb[:, 0:1], vb3[:, 0:1], cand[:, 0:1])
        nc.scalar.dma_start(out_dram.ap()[:, :], out_sb[:])

    nc.compile()
    return nc


def _get_nc():
    global _NC_CACHE
    if _NC_CACHE is None:
        _NC_CACHE = _build()
    return _NC_CACHE


def _shard(x: np.ndarray) -> list[dict[str, np.ndarray]]:
    xf = np.ascontiguousarray(np.asarray(x, dtype=np.float32).reshape(NK, ROW))
    ident = np.ascontiguousarray(np.eye(P, dtype=np.float32))
    riota_pc = np.zeros(512, np.float32)
    for c in range(C):
        for p in range(P):
            riota_pc[c * P + p] = S - (C * p + c)
    kiota = (float(S) * (np.arange(K) + 1)).astype(np.float32)
    consts = np.ascontiguousarray(np.concatenate(
        [np.tile(riota_pc, (K, 1)), kiota[:, None]], axis=1, dtype=np.float32))
    shards = []
    for c in range(N_CORES):
        lo = c * K
        s = xf[lo:min(lo + K, NK)]
        if s.shape[0] < K:
            s = np.concatenate(
                [s, np.zeros((K - s.shape[0], ROW), np.float32)], axis=0)
        shards.append({"x": np.ascontiguousarray(s),
                       "ident": ident, "consts": consts})
    return shards


def _run(x, trace=False, **kw):
    nc = _get_nc()
    res = run_bass_kernel_spmd(nc, _shard(x), core_ids=list(range(N_CORES)),
                               trace=trace, **kw)
    out = np.concatenate([r["out"] for r in res.results], axis=0)[:NK]
    return out.astype(np.float32), res


def kernel(x: np.ndarray) -> np.ndarray:
    out, _ = _run(x, trace=False)
    return out
